# revision 95
# baseline (speedup 1.0000x reference)
"""Trainium2 Bass kernel for nn_PartialRadialLayer.

Math (see reference):
  ang    = arccos(cos(x, ray)) / pi                       [B]
  dec_n  = sigmoid(alpha_n * ang + beta_n)                [B, 255]
  dist   = soft-bin products down the depth-8 tree        [B, 256]
  out    = einsum('bl,bi,liw->bw', dist, x, T)            [B, 32]

Key identity: out[b,:] = x[b,:] @ M(ang[b]) where M(a) = sum_l dist_l(a) T_l
is a smooth [I, W] matrix-valued function of the scalar angle. All tree
decisions are slope-6 sigmoids, so M is analytic in cos(pi*ang); a
degree-D Chebyshev expansion M ~ sum_d T_d(s) K_d with s an affine map of
cos(pi*ang) reaches ~1.7e-3 end-to-end at D=4 (gate is 2e-2).

Device strategy (pure data parallel over 8 cores, 8192 rows each):
  * s = dot/sqrt(ss*rn2*VH^2) - V0/VH; ss and dot by fp16 DVE
    multiply + pairwise-halving (2x mode) + reduce, chunked behind the
    x16 DMAs; Chebyshev recurrence on DVE; only sqrt rides ACT.
  * per 8-tile group: PE matmuls Q[b,(w,d)] = xT_tile.T @ Kall (K=64,
    W*D=128 cols, bank-aligned PSUM), one fused ACT PSUM->SBUF fp16
    cast, one fused DVE multiply by the per-row Chebyshev vector
    (broadcast over w) and two pairwise adds over d.
  * host precomputes K_d from the tree params only (alpha/beta/T/|ray|),
    ships x in two pure-layout fp16 forms (tiled and transposed), and
    un-permutes the fp16 output.
"""

import numpy as np

B = 65536
NCORES = 8
BC = B // NCORES          # 8192 rows per core
I = 64
W = 32
NT = BC // 128            # 64 batch tiles of 128 rows
D = 4                     # Chebyshev degree (terms)
V0, VH = 0.04, 0.56       # fixed Chebyshev domain in cos(pi*ang) units
DEPTH = 8
L = 256
EPS = 1e-8

# ----------------------------------------------------------------------------
# Environment workarounds (old walrus build in this image)
# ----------------------------------------------------------------------------

def _install_fixups():
    import orjson
    import concourse.tile as tile
    import concourse.mybir as mybir
    import concourse.bass2jax as bass2jax
    import concourse.bass_utils as bass_utils
    from concourse.vector_clock import ScopedClock

    if getattr(tile.TileContext, "_ant_fixups_installed", False):
        return

    # 1. Tail drain: at most one sync-wait per CTRL instruction.
    def _drain_and_barrier(self, tick_clock, wait_clock):
        drain_inst = self.nc.sync.drain()
        wait_clock.add_sem_waits(
            drain_inst.ins, ScopedClock({None: tick_clock.global_clock})
        )
        si = drain_inst.ins.sync_info
        waits = list(si.on_wait) if si is not None else []
        if len(waits) > 1:
            drain_inst.ins.sync_info = mybir.SyncInfo(
                on_wait=waits[:1], on_update=list(si.on_update)
            )
            for k in range(1, len(waits)):
                extra = self.nc.sync.drain()
                extra.ins.sync_info = mybir.SyncInfo(
                    on_wait=waits[k : k + 1], on_update=[]
                )
        self.nc.all_engine_barrier()
        popped = self.nc._tile_sem_poison_stack.pop()
        assert popped is self._sem_poison
        self.nc.clear_and_free_semaphores(list(self.sems.allocated().values()))
        self.nc.all_engine_barrier()

    tile.TileContext._drain_and_barrier = _drain_and_barrier
    tile.TileContext._ant_fixups_installed = True

    # 2. Split multi-wait instructions onto same-engine NoOps in the BIR.
    def _split_multiwait_bir(bir_bytes):
        d = orjson.loads(bir_bytes)
        for fn in d.get("functions", []):
            for blk in fn.get("blocks", []):
                out = []
                for inst in blk["instructions"]:
                    si = inst.get("sync_info")
                    waits = (si or {}).get("on_wait") or []
                    if len(waits) > 1 and inst.get("engine") not in (
                        None,
                        "Unassigned",
                    ):
                        for k, w in enumerate(waits[:-1]):
                            nop = {
                                "name": f"{inst['name']}-sw{k}",
                                "engine": inst["engine"],
                                "opcode": "NoOp",
                                "ins": [],
                                "outs": [],
                                "sync_info": {"on_wait": [w], "on_update": []},
                            }
                            if inst.get("debug") is not None:
                                nop["debug"] = inst["debug"]
                            out.append(nop)
                        si["on_wait"] = [waits[-1]]
                    out.append(inst)
                blk["instructions"] = out
        return orjson.dumps(d)

    orig = bass_utils.compile_bir_kernel

    def patched(bir_json, tmpdir, neff_name="file.neff"):
        return orig(_split_multiwait_bir(bytes(bir_json)), tmpdir, neff_name)

    bass_utils.compile_bir_kernel = patched
    bass2jax.compile_bir_kernel = patched


# ----------------------------------------------------------------------------
# Device program
# ----------------------------------------------------------------------------

_prog_cache = {}


def _build_program():
    if "nc" in _prog_cache:
        return _prog_cache["nc"]
    _install_fixups()
    import concourse.bass as bass
    import concourse.tile as tile
    import concourse.mybir as mybir

    f32, f16 = mybir.dt.float32, mybir.dt.float16
    AF = mybir.ActivationFunctionType
    ALU = mybir.AluOpType

    nc = bass.Bass("TRN2", target_bir_lowering=False, debug=False,
                   num_devices=NCORES)

    x16_d = nc.dram_tensor("x16", [128, NT * I], f16, kind="ExternalInput").ap()
    xt_d = nc.dram_tensor("xt16", [I, BC], f16, kind="ExternalInput").ap()
    kall_d = nc.dram_tensor("kall", [I, W * D], f16, kind="ExternalInput").ap()
    ray_d = nc.dram_tensor("ray16", [128, I], f16, kind="ExternalInput").ap()
    pp_d = nc.dram_tensor("pp", [128, 8], f32, kind="ExternalInput").ap()
    out_d = nc.dram_tensor("out16", [128, NT * W], f16,
                           kind="ExternalOutput").ap()

    # s = cos(pi*ang)/VH - V0/VH = dot/sqrt(ss*rn2*VH^2) - c0
    c0 = float(V0 / VH)

    with tile.TileContext(nc) as tc, nc.allow_low_precision(
        reason="fp16 reduce outputs; DVE accumulates wider internally"
    ):
        with (
            tc.tile_pool(name="const", bufs=1) as constp,
            tc.tile_pool(name="ph1", bufs=1) as ph1,
            tc.tile_pool(name="qpsum", bufs=2, space="PSUM") as qpsum,
            tc.tile_pool(name="qs", bufs=8) as qsp,
            tc.tile_pool(name="mq", bufs=4) as mqp,
            tc.tile_pool(name="mh", bufs=4) as mhp,
            tc.tile_pool(name="outp", bufs=4) as outp,
        ):
            # ---- inputs (split big loads across SP + ACT HWDGE queues) ----
            # ~75 GB/s per queue; uneven x16 chunks (first is small) so the
            # chunked phase-1 head starts as early as possible.
            CHUNKS = [(0, 8), (8, 8), (16, 16), (32, 16), (48, 16)]
            x16 = constp.tile([128, NT * I], f16, tag="x16")
            ray = constp.tile([128, I], f16, tag="ray")
            pp = constp.tile([128, 8], f32, tag="pp")
            kall = constp.tile([I, W * D], f16, tag="kall")
            xt = constp.tile([I, BC], f16, tag="xt")
            # x16c0 leads the SP queue; only ray (needed by the first xr)
            # rides ACT's queue head — pp defers with the first xt batch.
            nc.scalar.dma_start(ray[:], ray_d[:])
            for c, (ct0, cn) in enumerate(CHUNKS):
                eng = nc.sync if c % 2 == 0 else nc.scalar
                eng.dma_start(
                    x16[:, ct0 * I : (ct0 + cn) * I],
                    x16_d[:, ct0 * I : (ct0 + cn) * I],
                )
            nc.sync.dma_start(kall[:], kall_d[:])
            # xt: SP-queue chunks dispatch now; ACT-queue chunks are emitted
            # later (after the first phase-1 squares) so the ACT dispatch
            # overhead doesn't delay the squares that gate the DVE pipeline.
            for c in range(0, 8, 2):
                nc.sync.dma_start(
                    xt[:, c * (BC // 8) : (c + 1) * (BC // 8)],
                    xt_d[:, c * (BC // 8) : (c + 1) * (BC // 8)],
                )

            def emit_xt_act(cs, with_pp=False):
                if with_pp:
                    nc.scalar.dma_start(pp[:], pp_d[:])
                for c in cs:
                    nc.scalar.dma_start(
                        xt[:, c * (BC // 8) : (c + 1) * (BC // 8)],
                        xt_d[:, c * (BC // 8) : (c + 1) * (BC // 8)],
                    )

            # ---- phase 1 state ----
            xboth = ph1.tile([128, 2 * NT * I], f16, tag="xboth")
            st = ph1.tile([128, 8 * NT], f16, tag="st")
            ss = st[:, 0 * NT : 1 * NT]
            dot = st[:, 1 * NT : 2 * NT]
            sq = st[:, 2 * NT : 3 * NT]
            rsq = st[:, 4 * NT : 5 * NT]
            v = st[:, 5 * NT : 6 * NT]
            hb1 = ph1.tile([128, NT * I], f16, tag="hb1")
            hb2 = ph1.tile([128, NT * I // 2], f16, tag="hb2")
            hb3 = ph1.tile([128, NT * I // 4], f16, tag="hb3")
            xb4 = xboth[:].rearrange("p (k t i) -> p k t i", k=2, i=I)
            h1v = hb1[:].rearrange("p (k t i) -> p k t i", k=2, i=I // 2)
            h2v = hb2[:].rearrange("p (k t i) -> p k t i", k=2, i=I // 4)
            h3v = hb3[:].rearrange("p (k t i) -> p k t i", k=2, i=I // 8)
            stv = st[:, 0 : 2 * NT].rearrange("p (k t) -> p k t", k=2)
            cheb = ph1.tile([128, NT * D], f16, tag="cheb")
            cheb3 = cheb[:].rearrange("p (t d) -> p t d", d=D)
            tmp = ph1.tile([128, 2 * NT], f16, tag="tmp")
            # T0 = 1 is constant: write it early on the idle GpSimd engine
            nc.gpsimd.memset(cheb3[:, :, 0], 1.0)

            def emit_p1(ct0, cn):
                """x^2 | x*ray, pairwise-halve, reduce into ss/dot.
                Squares ride the otherwise-idle ACT engine."""
                ts_ = slice(ct0, ct0 + cn)
                xc = x16[:, ct0 * I : (ct0 + cn) * I].rearrange(
                    "p (t i) -> p t i", i=I
                )
                nc.scalar.activation(
                    xboth[:, ct0 * I : (ct0 + cn) * I],
                    x16[:, ct0 * I : (ct0 + cn) * I], AF.Square,
                )
                nc.vector.tensor_mul(
                    xb4[:, 1, ts_, :], xc,
                    ray[:].unsqueeze(1).broadcast_to((128, cn, I)),
                )
                nc.vector.tensor_add(
                    h1v[:, :, ts_, :],
                    xb4[:, :, ts_, 0 : I // 2], xb4[:, :, ts_, I // 2 : I],
                )
                nc.vector.tensor_add(
                    h2v[:, :, ts_, :],
                    h1v[:, :, ts_, 0 : I // 4],
                    h1v[:, :, ts_, I // 4 : I // 2],
                )
                nc.vector.tensor_add(
                    h3v[:, :, ts_, :],
                    h2v[:, :, ts_, 0 : I // 8],
                    h2v[:, :, ts_, I // 8 : I // 4],
                )
                nc.vector.reduce_sum(
                    stv[:, :, ts_], h3v[:, :, ts_, :],
                    axis=mybir.AxisListType.X,
                )
            def emit_chain(t0, tn):
                """s = dot/sqrt(ss*rn2*VH^2) - c0, then cheb recurrence."""
                r = slice(t0, t0 + tn)
                nc.scalar.activation(
                    sq[:, r], ss[:, r], AF.Sqrt, scale=pp[:, 1:2]
                )
                nc.vector.reciprocal(rsq[:, r], sq[:, r])
                nc.vector.tensor_mul(v[:, r], dot[:, r], rsq[:, r])
                nc.vector.tensor_scalar(
                    cheb3[:, r, 1], v[:, r], 1.0, -c0,
                    op0=ALU.mult, op1=ALU.add,
                )
                for dd in range(2, D):
                    tslot = tmp[:, (dd % 2) * NT + t0 :
                                 (dd % 2) * NT + t0 + tn]
                    # tslot = (T_{d-1} * 2) * s
                    nc.vector.scalar_tensor_tensor(
                        tslot, cheb3[:, r, dd - 1], 2.0, cheb3[:, r, 1],
                        op0=ALU.mult, op1=ALU.mult,
                    )
                    nc.vector.tensor_sub(
                        cheb3[:, r, dd], tslot, cheb3[:, r, dd - 2]
                    )

            # ---- phase 2: per 8-tile group, Q matmuls + fused contraction ----
            # W*D = 128 fp32 cols = 512B per tile: 4 matmul outputs per PSUM
            # bank, no bank-boundary crossings, contiguous group layout.
            F = 16                  # tiles fused per ACT/DVE op + per DMA

            def emit_group(g):
                t0 = g * F
                qp = qpsum.tile([128, F * W * D], f32, tag="qp")
                for k in range(F):
                    nc.tensor.matmul(
                        qp[:, k * W * D : (k + 1) * W * D],
                        xt[:, (t0 + k) * 128 : (t0 + k + 1) * 128],
                        kall[:], start=True, stop=True,
                    )
                qs = qsp.tile([128, F * W * D], f16, tag="qs")
                nc.scalar.activation(qs[:], qp[:], AF.Copy)
                mq = mqp.tile([128, F * W * D], f16, tag="mq")
                nc.vector.tensor_mul(
                    mq[:].rearrange("p (t w d) -> p t w d", w=W, d=D),
                    qs[:].rearrange("p (t w d) -> p t w d", w=W, d=D),
                    cheb3[:, t0 : t0 + F, :].unsqueeze(2).broadcast_to(
                        (128, F, W, D)
                    ),
                )
                # pairwise d-reduction: first step runs in 2x mode
                mh = mhp.tile([128, F * W * 2], f16, tag="mh")
                mq4 = mq[:].rearrange("p (tw d) -> p tw d", d=D)
                mh2 = mh[:].rearrange("p (tw d) -> p tw d", d=2)
                nc.vector.tensor_add(mh2, mq4[:, :, 0:2], mq4[:, :, 2:4])
                outg = outp.tile([128, F * W], f16, tag="outg")
                nc.vector.tensor_add(
                    outg[:].unsqueeze(2), mh2[:, :, 0:1], mh2[:, :, 1:2],
                )
                if g == NT // F - 1:
                    # final group: halves on both queues so the last transfer
                    # in the tail is half as long
                    HF = F * W // 2
                    nc.sync.dma_start(
                        out_d[:, g * F * W : g * F * W + HF], outg[:, 0:HF]
                    )
                    nc.scalar.dma_start(
                        out_d[:, g * F * W + HF : (g + 1) * F * W],
                        outg[:, HF:],
                    )
                else:
                    nc.sync.dma_start(
                        out_d[:, g * F * W : (g + 1) * F * W], outg[:]
                    )

            # ---- sequential phases; the tile scheduler handles overlap ----
            for ci, (ct0, cn) in enumerate(CHUNKS):
                emit_p1(ct0, cn)
                if ci == 1:
                    emit_xt_act((1, 3), with_pp=True)
            emit_xt_act((5, 7))
            emit_chain(0, NT)
            for g in range(NT // F):
                emit_group(g)

    _prog_cache["nc"] = nc
    return nc


# ----------------------------------------------------------------------------
# Host wrapper
# ----------------------------------------------------------------------------

def _tree_paths(depth):
    node_idx = np.zeros((2**depth, depth), dtype=np.int64)
    is_right = np.zeros((2**depth, depth), dtype=bool)
    for leaf in range(2**depth):
        idx = 0
        for level in range(depth):
            bit = (leaf >> (depth - 1 - level)) & 1
            node_idx[leaf, level] = idx
            is_right[leaf, level] = bool(bit)
            idx = 2 * idx + 1 + bit
    return node_idx, is_right


def _host_prep(x, ray, inner_transforms, w_i, b_i, a_i):
    x = np.asarray(x, dtype=np.float32)
    ray = np.asarray(ray, dtype=np.float64)
    T = np.asarray(inner_transforms, dtype=np.float64)
    w_i = np.asarray(w_i, dtype=np.float64)
    b_i = np.asarray(b_i, dtype=np.float64)
    a_i = np.asarray(a_i, dtype=np.float64)

    def sig(z):
        return 1.0 / (1.0 + np.exp(-z))

    alpha = ((0.5 + sig(w_i)) * (1.0 + a_i))[0]      # [255]
    beta = (-sig(b_i) * (1.0 + a_i))[0]              # [255]
    node_idx, is_right = _tree_paths(DEPTH)

    def dist_of_a(a):
        dec = sig(a[:, None] * alpha[None, :] + beta[None, :])
        g = dec[:, node_idx]
        return np.prod(np.where(is_right[None], 1.0 - g, g), axis=2)

    # Chebyshev interpolation of M(cos(pi*ang)) at D nodes on V0 +/- VH
    kk = np.arange(D)
    theta = np.pi * (kk + 0.5) / D
    cnodes = V0 + np.cos(theta) * VH
    anodes = np.arccos(np.clip(cnodes, -1.0, 1.0)) / np.pi
    Mnodes = dist_of_a(anodes) @ T.reshape(L, I * W)        # [D, I*W]
    Cmat = np.cos(np.outer(kk, theta))                      # [D, D]
    coef = (2.0 / D) * (Cmat @ Mnodes)
    coef[0] *= 0.5
    K = coef.reshape(D, I, W)
    # kall[i, w*D + d] = K[d, i, w]
    kall = np.ascontiguousarray(K.transpose(1, 2, 0).reshape(I, W * D)
                                ).astype(np.float16)

    rn = max(float(np.linalg.norm(ray[0])), EPS)
    pp = np.zeros((128, 8), dtype=np.float32)
    pp[:, 0] = rn * rn
    pp[:, 1] = rn * rn * VH * VH

    ray16 = np.tile(ray[0].astype(np.float16), (128, 1))    # [128, I]
    x16 = x.astype(np.float16)
    return x16, kall, ray16, pp


def _in_maps(x16, kall, ray16, pp):
    maps = []
    for cid in range(NCORES):
        xc = x16[cid * BC : (cid + 1) * BC]                 # [BC, I]
        x16l = np.ascontiguousarray(
            xc.reshape(NT, 128, I).transpose(1, 0, 2).reshape(128, NT * I)
        )
        xt16 = np.ascontiguousarray(xc.T)                   # [I, BC]
        maps.append({
            "x16": x16l,
            "xt16": xt16,
            "kall": kall,
            "ray16": ray16,
            "pp": pp,
        })
    return maps


def _gather_out(res):
    outs = []
    for c in range(NCORES):
        o = res.results[c]["out16"]                         # [128, NT*W] f16
        outs.append(
            o.reshape(128, NT, W).transpose(1, 0, 2).reshape(BC, W)
        )
    return np.concatenate(outs, axis=0).astype(np.float32)


def kernel(x, ray, inner_transforms, w_i, b_i, a_i):
    from concourse.bass_utils import run_bass_kernel_spmd

    prep = _host_prep(x, ray, inner_transforms, w_i, b_i, a_i)
    nc = _build_program()
    res = run_bass_kernel_spmd(nc, _in_maps(*prep),
                               core_ids=list(range(NCORES)))
    return _gather_out(res)


def run_traced(inputs):
    """For test.py: same as kernel() but with NTFF tracing; returns
    (output, BassKernelResults)."""
    from concourse.bass_utils import run_bass_kernel_spmd

    prep = _host_prep(**inputs)
    nc = _build_program()
    res = run_bass_kernel_spmd(
        nc, _in_maps(*prep), core_ids=list(range(NCORES)), trace=True
    )
    return _gather_out(res), res


# revision 96
# speedup vs baseline: 1.0744x; 1.0744x over previous
"""Trainium2 Bass kernel for nn_PartialRadialLayer.

Math (see reference):
  ang    = arccos(cos(x, ray)) / pi                       [B]
  dec_n  = sigmoid(alpha_n * ang + beta_n)                [B, 255]
  dist   = soft-bin products down the depth-8 tree        [B, 256]
  out    = einsum('bl,bi,liw->bw', dist, x, T)            [B, 32]

Key identity: out[b,:] = x[b,:] @ M(ang[b]) where M(a) = sum_l dist_l(a) T_l
is a smooth [I, W] matrix-valued function of the scalar angle. All tree
decisions are slope-6 sigmoids, so M is analytic in cos(pi*ang); a
degree-D Chebyshev expansion M ~ sum_d T_d(s) K_d with s an affine map of
cos(pi*ang) reaches ~1.7e-3 end-to-end at D=4 (gate is 2e-2).

Device strategy (pure data parallel over 8 cores, 8192 rows each):
  * s = dot/sqrt(ss*rn2*VH^2) - V0/VH; ss and dot by fp16 DVE
    multiply + pairwise-halving (2x mode) + reduce, chunked behind the
    x16 DMAs; Chebyshev recurrence on DVE; only sqrt rides ACT.
  * per 8-tile group: PE matmuls Q[b,(w,d)] = xT_tile.T @ Kall (K=64,
    W*D=128 cols, bank-aligned PSUM), one fused ACT PSUM->SBUF fp16
    cast, one fused DVE multiply by the per-row Chebyshev vector
    (broadcast over w) and two pairwise adds over d.
  * host precomputes K_d from the tree params only (alpha/beta/T/|ray|),
    ships x in two pure-layout fp16 forms (tiled and transposed), and
    un-permutes the fp16 output.
"""

import numpy as np

B = 65536
NCORES = 8
BC = B // NCORES          # 8192 rows per core
I = 64
W = 32
NT = BC // 128            # 64 batch tiles of 128 rows
D = 4                     # Chebyshev degree (terms)
V0, VH = 0.04, 0.56       # fixed Chebyshev domain in cos(pi*ang) units
DEPTH = 8
L = 256
EPS = 1e-8

# ----------------------------------------------------------------------------
# Environment workarounds (old walrus build in this image)
# ----------------------------------------------------------------------------

def _install_fixups():
    import orjson
    import concourse.tile as tile
    import concourse.mybir as mybir
    import concourse.bass2jax as bass2jax
    import concourse.bass_utils as bass_utils
    from concourse.vector_clock import ScopedClock

    if getattr(tile.TileContext, "_ant_fixups_installed", False):
        return

    # 1. Tail drain: at most one sync-wait per CTRL instruction.
    def _drain_and_barrier(self, tick_clock, wait_clock):
        drain_inst = self.nc.sync.drain()
        wait_clock.add_sem_waits(
            drain_inst.ins, ScopedClock({None: tick_clock.global_clock})
        )
        si = drain_inst.ins.sync_info
        waits = list(si.on_wait) if si is not None else []
        if len(waits) > 1:
            drain_inst.ins.sync_info = mybir.SyncInfo(
                on_wait=waits[:1], on_update=list(si.on_update)
            )
            for k in range(1, len(waits)):
                extra = self.nc.sync.drain()
                extra.ins.sync_info = mybir.SyncInfo(
                    on_wait=waits[k : k + 1], on_update=[]
                )
        self.nc.all_engine_barrier()
        popped = self.nc._tile_sem_poison_stack.pop()
        assert popped is self._sem_poison
        self.nc.clear_and_free_semaphores(list(self.sems.allocated().values()))
        self.nc.all_engine_barrier()

    tile.TileContext._drain_and_barrier = _drain_and_barrier
    tile.TileContext._ant_fixups_installed = True

    # 2. Split multi-wait instructions onto same-engine NoOps in the BIR.
    def _split_multiwait_bir(bir_bytes):
        d = orjson.loads(bir_bytes)
        for fn in d.get("functions", []):
            for blk in fn.get("blocks", []):
                out = []
                for inst in blk["instructions"]:
                    si = inst.get("sync_info")
                    waits = (si or {}).get("on_wait") or []
                    if len(waits) > 1 and inst.get("engine") not in (
                        None,
                        "Unassigned",
                    ):
                        for k, w in enumerate(waits[:-1]):
                            nop = {
                                "name": f"{inst['name']}-sw{k}",
                                "engine": inst["engine"],
                                "opcode": "NoOp",
                                "ins": [],
                                "outs": [],
                                "sync_info": {"on_wait": [w], "on_update": []},
                            }
                            if inst.get("debug") is not None:
                                nop["debug"] = inst["debug"]
                            out.append(nop)
                        si["on_wait"] = [waits[-1]]
                    out.append(inst)
                blk["instructions"] = out
        return orjson.dumps(d)

    orig = bass_utils.compile_bir_kernel

    def patched(bir_json, tmpdir, neff_name="file.neff"):
        return orig(_split_multiwait_bir(bytes(bir_json)), tmpdir, neff_name)

    bass_utils.compile_bir_kernel = patched
    bass2jax.compile_bir_kernel = patched


# ----------------------------------------------------------------------------
# Device program
# ----------------------------------------------------------------------------

_prog_cache = {}


def _build_program():
    if "nc" in _prog_cache:
        return _prog_cache["nc"]
    _install_fixups()
    import concourse.bass as bass
    import concourse.tile as tile
    import concourse.mybir as mybir

    f32, f16 = mybir.dt.float32, mybir.dt.float16
    AF = mybir.ActivationFunctionType
    ALU = mybir.AluOpType

    nc = bass.Bass("TRN2", target_bir_lowering=False, debug=False,
                   num_devices=NCORES)

    x16_d = nc.dram_tensor("x16", [128, NT * I], f16, kind="ExternalInput").ap()
    xt_d = nc.dram_tensor("xt16", [I, BC], f16, kind="ExternalInput").ap()
    kall_d = nc.dram_tensor("kall", [I, W * D], f16, kind="ExternalInput").ap()
    ray_d = nc.dram_tensor("ray16", [128, I], f16, kind="ExternalInput").ap()
    pp_d = nc.dram_tensor("pp", [128, 8], f32, kind="ExternalInput").ap()
    out_d = nc.dram_tensor("out16", [128, NT * W], f16,
                           kind="ExternalOutput").ap()

    # s = cos(pi*ang)/VH - V0/VH = dot/sqrt(ss*rn2*VH^2) - c0
    c0 = float(V0 / VH)

    with tile.TileContext(nc) as tc, nc.allow_low_precision(
        reason="fp16 reduce outputs; DVE accumulates wider internally"
    ):
        with (
            tc.tile_pool(name="const", bufs=1) as constp,
            tc.tile_pool(name="ph1", bufs=1) as ph1,
            tc.tile_pool(name="qpsum", bufs=2, space="PSUM") as qpsum,
            tc.tile_pool(name="qs", bufs=8) as qsp,
            tc.tile_pool(name="mq", bufs=4) as mqp,
            tc.tile_pool(name="mh", bufs=4) as mhp,
            tc.tile_pool(name="outp", bufs=4) as outp,
        ):
            # ---- inputs (split big loads across SP + ACT HWDGE queues) ----
            # ~75 GB/s per queue; uneven x16 chunks (first is small) so the
            # chunked phase-1 head starts as early as possible.
            CHUNKS = [(0, 8), (8, 16), (24, 16), (40, 16), (56, 8)]
            x16 = constp.tile([128, NT * I], f16, tag="x16")
            ray = constp.tile([128, I], f16, tag="ray")
            pp = constp.tile([128, 8], f32, tag="pp")
            kall = constp.tile([I, W * D], f16, tag="kall")
            xt = constp.tile([I, BC], f16, tag="xt")
            # x16c0 leads the SP queue; only ray (needed by the first xr)
            # rides ACT's queue head — pp defers with the first xt batch.
            nc.scalar.dma_start(ray[:], ray_d[:])
            for c, (ct0, cn) in enumerate(CHUNKS):
                eng = nc.sync if c % 2 == 0 else nc.scalar
                eng.dma_start(
                    x16[:, ct0 * I : (ct0 + cn) * I],
                    x16_d[:, ct0 * I : (ct0 + cn) * I],
                )
            nc.sync.dma_start(kall[:], kall_d[:])
            # xt: SP-queue chunks dispatch now; ACT-queue chunks are emitted
            # later (after the first phase-1 squares) so the ACT dispatch
            # overhead doesn't delay the squares that gate the DVE pipeline.
            for c in range(0, 8, 2):
                nc.sync.dma_start(
                    xt[:, c * (BC // 8) : (c + 1) * (BC // 8)],
                    xt_d[:, c * (BC // 8) : (c + 1) * (BC // 8)],
                )

            def emit_xt_act(cs, with_pp=False):
                if with_pp:
                    nc.scalar.dma_start(pp[:], pp_d[:])
                for c in cs:
                    nc.scalar.dma_start(
                        xt[:, c * (BC // 8) : (c + 1) * (BC // 8)],
                        xt_d[:, c * (BC // 8) : (c + 1) * (BC // 8)],
                    )

            # ---- phase 1 state ----
            xboth = ph1.tile([128, 2 * NT * I], f16, tag="xboth")
            st = ph1.tile([128, 8 * NT], f16, tag="st")
            ss = st[:, 0 * NT : 1 * NT]
            dot = st[:, 1 * NT : 2 * NT]
            sq = st[:, 2 * NT : 3 * NT]
            rsq = st[:, 4 * NT : 5 * NT]
            v = st[:, 5 * NT : 6 * NT]
            hb1 = ph1.tile([128, NT * I], f16, tag="hb1")
            hb2 = ph1.tile([128, NT * I // 2], f16, tag="hb2")
            hb3 = ph1.tile([128, NT * I // 4], f16, tag="hb3")
            xb4 = xboth[:].rearrange("p (k t i) -> p k t i", k=2, i=I)
            h1v = hb1[:].rearrange("p (k t i) -> p k t i", k=2, i=I // 2)
            h2v = hb2[:].rearrange("p (k t i) -> p k t i", k=2, i=I // 4)
            h3v = hb3[:].rearrange("p (k t i) -> p k t i", k=2, i=I // 8)
            stv = st[:, 0 : 2 * NT].rearrange("p (k t) -> p k t", k=2)
            cheb = ph1.tile([128, NT * D], f16, tag="cheb")
            cheb3 = cheb[:].rearrange("p (t d) -> p t d", d=D)
            tmp = ph1.tile([128, 2 * NT], f16, tag="tmp")
            # T0 = 1 is constant: write it early on the idle GpSimd engine
            nc.gpsimd.memset(cheb3[:, :, 0], 1.0)

            def emit_p1(ct0, cn):
                """x^2 | x*ray, pairwise-halve, reduce into ss/dot.
                Squares ride the otherwise-idle ACT engine."""
                ts_ = slice(ct0, ct0 + cn)
                xc = x16[:, ct0 * I : (ct0 + cn) * I].rearrange(
                    "p (t i) -> p t i", i=I
                )
                nc.scalar.activation(
                    xboth[:, ct0 * I : (ct0 + cn) * I],
                    x16[:, ct0 * I : (ct0 + cn) * I], AF.Square,
                )
                nc.vector.tensor_mul(
                    xb4[:, 1, ts_, :], xc,
                    ray[:].unsqueeze(1).broadcast_to((128, cn, I)),
                )
                nc.vector.tensor_add(
                    h1v[:, :, ts_, :],
                    xb4[:, :, ts_, 0 : I // 2], xb4[:, :, ts_, I // 2 : I],
                )
                nc.vector.tensor_add(
                    h2v[:, :, ts_, :],
                    h1v[:, :, ts_, 0 : I // 4],
                    h1v[:, :, ts_, I // 4 : I // 2],
                )
                nc.vector.tensor_add(
                    h3v[:, :, ts_, :],
                    h2v[:, :, ts_, 0 : I // 8],
                    h2v[:, :, ts_, I // 8 : I // 4],
                )
                nc.vector.reduce_sum(
                    stv[:, :, ts_], h3v[:, :, ts_, :],
                    axis=mybir.AxisListType.X,
                )
            def emit_chain(t0, tn):
                """s = dot/sqrt(ss*rn2*VH^2) - c0, then cheb recurrence."""
                r = slice(t0, t0 + tn)
                nc.scalar.activation(
                    sq[:, r], ss[:, r], AF.Sqrt, scale=pp[:, 1:2]
                )
                nc.vector.reciprocal(rsq[:, r], sq[:, r])
                nc.vector.tensor_mul(v[:, r], dot[:, r], rsq[:, r])
                nc.vector.tensor_scalar(
                    cheb3[:, r, 1], v[:, r], 1.0, -c0,
                    op0=ALU.mult, op1=ALU.add,
                )
                for dd in range(2, D):
                    tslot = tmp[:, (dd % 2) * NT + t0 :
                                 (dd % 2) * NT + t0 + tn]
                    # tslot = (T_{d-1} * 2) * s
                    nc.vector.scalar_tensor_tensor(
                        tslot, cheb3[:, r, dd - 1], 2.0, cheb3[:, r, 1],
                        op0=ALU.mult, op1=ALU.mult,
                    )
                    nc.vector.tensor_sub(
                        cheb3[:, r, dd], tslot, cheb3[:, r, dd - 2]
                    )

            # ---- phase 2: per 8-tile group, Q matmuls + fused contraction ----
            # W*D = 128 fp32 cols = 512B per tile: 4 matmul outputs per PSUM
            # bank, no bank-boundary crossings, contiguous group layout.
            F = 16                  # tiles fused per ACT/DVE op + per DMA

            def emit_group(g):
                t0 = g * F
                qp = qpsum.tile([128, F * W * D], f32, tag="qp")
                for k in range(F):
                    nc.tensor.matmul(
                        qp[:, k * W * D : (k + 1) * W * D],
                        xt[:, (t0 + k) * 128 : (t0 + k + 1) * 128],
                        kall[:], start=True, stop=True,
                    )
                qs = qsp.tile([128, F * W * D], f16, tag="qs")
                nc.scalar.activation(qs[:], qp[:], AF.Copy)
                mq = mqp.tile([128, F * W * D], f16, tag="mq")
                nc.vector.tensor_mul(
                    mq[:].rearrange("p (t w d) -> p t w d", w=W, d=D),
                    qs[:].rearrange("p (t w d) -> p t w d", w=W, d=D),
                    cheb3[:, t0 : t0 + F, :].unsqueeze(2).broadcast_to(
                        (128, F, W, D)
                    ),
                )
                # pairwise d-reduction: first step runs in 2x mode
                mh = mhp.tile([128, F * W * 2], f16, tag="mh")
                mq4 = mq[:].rearrange("p (tw d) -> p tw d", d=D)
                mh2 = mh[:].rearrange("p (tw d) -> p tw d", d=2)
                nc.vector.tensor_add(mh2, mq4[:, :, 0:2], mq4[:, :, 2:4])
                outg = outp.tile([128, F * W], f16, tag="outg")
                nc.vector.tensor_add(
                    outg[:].unsqueeze(2), mh2[:, :, 0:1], mh2[:, :, 1:2],
                )
                if g == NT // F - 1:
                    # final group: halves on both queues so the last transfer
                    # in the tail is half as long
                    HF = F * W // 2
                    nc.sync.dma_start(
                        out_d[:, g * F * W : g * F * W + HF], outg[:, 0:HF]
                    )
                    nc.scalar.dma_start(
                        out_d[:, g * F * W + HF : (g + 1) * F * W],
                        outg[:, HF:],
                    )
                else:
                    nc.sync.dma_start(
                        out_d[:, g * F * W : (g + 1) * F * W], outg[:]
                    )

            # ---- sequential phases; the tile scheduler handles overlap ----
            for ci, (ct0, cn) in enumerate(CHUNKS):
                emit_p1(ct0, cn)
                if ci == 1:
                    emit_xt_act((1, 3), with_pp=True)
            emit_xt_act((5, 7))
            emit_chain(0, NT)
            for g in range(NT // F):
                emit_group(g)

    _prog_cache["nc"] = nc
    return nc


# ----------------------------------------------------------------------------
# Host wrapper
# ----------------------------------------------------------------------------

def _tree_paths(depth):
    node_idx = np.zeros((2**depth, depth), dtype=np.int64)
    is_right = np.zeros((2**depth, depth), dtype=bool)
    for leaf in range(2**depth):
        idx = 0
        for level in range(depth):
            bit = (leaf >> (depth - 1 - level)) & 1
            node_idx[leaf, level] = idx
            is_right[leaf, level] = bool(bit)
            idx = 2 * idx + 1 + bit
    return node_idx, is_right


def _host_prep(x, ray, inner_transforms, w_i, b_i, a_i):
    x = np.asarray(x, dtype=np.float32)
    ray = np.asarray(ray, dtype=np.float64)
    T = np.asarray(inner_transforms, dtype=np.float64)
    w_i = np.asarray(w_i, dtype=np.float64)
    b_i = np.asarray(b_i, dtype=np.float64)
    a_i = np.asarray(a_i, dtype=np.float64)

    def sig(z):
        return 1.0 / (1.0 + np.exp(-z))

    alpha = ((0.5 + sig(w_i)) * (1.0 + a_i))[0]      # [255]
    beta = (-sig(b_i) * (1.0 + a_i))[0]              # [255]
    node_idx, is_right = _tree_paths(DEPTH)

    def dist_of_a(a):
        dec = sig(a[:, None] * alpha[None, :] + beta[None, :])
        g = dec[:, node_idx]
        return np.prod(np.where(is_right[None], 1.0 - g, g), axis=2)

    # Chebyshev interpolation of M(cos(pi*ang)) at D nodes on V0 +/- VH
    kk = np.arange(D)
    theta = np.pi * (kk + 0.5) / D
    cnodes = V0 + np.cos(theta) * VH
    anodes = np.arccos(np.clip(cnodes, -1.0, 1.0)) / np.pi
    Mnodes = dist_of_a(anodes) @ T.reshape(L, I * W)        # [D, I*W]
    Cmat = np.cos(np.outer(kk, theta))                      # [D, D]
    coef = (2.0 / D) * (Cmat @ Mnodes)
    coef[0] *= 0.5
    K = coef.reshape(D, I, W)
    # kall[i, w*D + d] = K[d, i, w]
    kall = np.ascontiguousarray(K.transpose(1, 2, 0).reshape(I, W * D)
                                ).astype(np.float16)

    rn = max(float(np.linalg.norm(ray[0])), EPS)
    pp = np.zeros((128, 8), dtype=np.float32)
    pp[:, 0] = rn * rn
    pp[:, 1] = rn * rn * VH * VH

    ray16 = np.tile(ray[0].astype(np.float16), (128, 1))    # [128, I]
    x16 = x.astype(np.float16)
    return x16, kall, ray16, pp


def _in_maps(x16, kall, ray16, pp):
    maps = []
    for cid in range(NCORES):
        xc = x16[cid * BC : (cid + 1) * BC]                 # [BC, I]
        x16l = np.ascontiguousarray(
            xc.reshape(NT, 128, I).transpose(1, 0, 2).reshape(128, NT * I)
        )
        xt16 = np.ascontiguousarray(xc.T)                   # [I, BC]
        maps.append({
            "x16": x16l,
            "xt16": xt16,
            "kall": kall,
            "ray16": ray16,
            "pp": pp,
        })
    return maps


def _gather_out(res):
    outs = []
    for c in range(NCORES):
        o = res.results[c]["out16"]                         # [128, NT*W] f16
        outs.append(
            o.reshape(128, NT, W).transpose(1, 0, 2).reshape(BC, W)
        )
    return np.concatenate(outs, axis=0).astype(np.float32)


def kernel(x, ray, inner_transforms, w_i, b_i, a_i):
    from concourse.bass_utils import run_bass_kernel_spmd

    prep = _host_prep(x, ray, inner_transforms, w_i, b_i, a_i)
    nc = _build_program()
    res = run_bass_kernel_spmd(nc, _in_maps(*prep),
                               core_ids=list(range(NCORES)))
    return _gather_out(res)


def run_traced(inputs):
    """For test.py: same as kernel() but with NTFF tracing; returns
    (output, BassKernelResults)."""
    from concourse.bass_utils import run_bass_kernel_spmd

    prep = _host_prep(**inputs)
    nc = _build_program()
    res = run_bass_kernel_spmd(
        nc, _in_maps(*prep), core_ids=list(range(NCORES)), trace=True
    )
    return _gather_out(res), res
